# revision 21
# baseline (speedup 1.0000x reference)
"""CyclicVQ forward for Trainium2 (Bass, raw multi-engine pipeline, 8 cores).

Math: for each of 3 channels with n bins uniformly covering [-pi, pi), the
geodesic argmin over bin centers reduces to idx = rint(a*s + t) with
s = n/(2*pi), t = pi*s - 0.5 (f32 two-RN, matching the reference's decision
boundaries to within ~1 ulp).  quantized = centers[idx] via a fused ACT
affine (FMA) from the int index tile.  Null masking is fused
scalar_tensor_tensor ops: q *= (m == 0), i = max(i, m * n_bins).
A tiny host-side patch recomputes the exact reference semantics (f32
distance argmin) for the ~2k elements within 2e-5 of an ideal bin boundary,
where ulp-level rounding differences between the shortcut and the
reference's distance computation can flip the argmin.  A host `q += 0.0`
normalizes the -0.0 produced by masking negative q values.

Per-core pipeline (memory-bound; DMA ~13.6us per 1024-position chunk):
  SP:  load angles/mask chunks, store q/idx chunks (per-slot DMA sems)
  DVE: u' = a*s + t (3 strided fused TS), then masking (4 strided STT)
  ACT: i = rint(u') (contiguous convert), q = i*w + b (3 strided FMA)

Sharding: pure data parallel over the leading batch dim (4096 -> 8 x 512).
"""
import sys

sys.path.insert(0, "/opt/trn_rl_repo")

from contextlib import ExitStack

import numpy as np

import concourse.bass as bass
import concourse.mybir as mybir
from concourse.bass_utils import run_bass_kernel_spmd

# ---------------------------------------------------------------- constants
N_BINS = (24, 12, 16)
N_CORES = 8
B0, B1, B2 = 4096, 2048, 3  # angles shape
ROWS_PER_CORE = B0 // N_CORES  # 512
POS_PER_CORE = ROWS_PER_CORE * B1  # 1,048,576 positions
P = 128  # partitions
POS_PER_PART = POS_PER_CORE // P  # 8192
N_CHUNKS = 8
T = POS_PER_PART // N_CHUNKS  # 1024 positions / partition / chunk
NB = 6  # buffer slots (26KB SBUF per slot)
        # ~4 chunks, well past the ~25us per-chunk pipeline latency)

F32 = mybir.dt.float32
I32 = mybir.dt.int32
U8 = mybir.dt.uint8
ALU = mybir.AluOpType
ACT_COPY = mybir.ActivationFunctionType.Copy

_PI64 = np.float64(np.pi)
# per-channel device constants (f32, host-rounded)
_S = [np.float32(n / (2 * np.pi)) for n in N_BINS]  # u' = a*s + t
_T = [np.float32(_PI64 * np.float64(s) - 0.5) for n, s in zip(N_BINS, _S)]
_W = [np.float32(2 * np.pi / n) for n in N_BINS]  # center = i*w + b (FMA)
_B = [np.float32(0.5 * np.float64(w) - _PI64) for w in _W]

_PATCH_DELTA = 2e-5  # host-patch window around ideal boundaries (radians)

_NC_CACHE = None


def _build_nc():
    """Build the per-core Bass program (identical on all 8 cores)."""
    nc = bass.Bass()

    FE = POS_PER_PART * 3  # 24576 f32 per partition
    FM = POS_PER_PART * 2  # 16384 u8 per partition

    ang = nc.dram_tensor("angles", [P, FE], F32, kind="ExternalInput")
    msk = nc.dram_tensor("null_mask", [P, FM], U8, kind="ExternalInput")
    oq = nc.dram_tensor("q", [P, FE], F32, kind="ExternalOutput")
    oi = nc.dram_tensor("idx", [P, FE], I32, kind="ExternalOutput")

    with ExitStack() as ctx:
        # a_sb holds angles, then u' in place, then q (ACT writes centers
        # over the dead u') -- one f32 tile per slot instead of two.
        a_sb = ctx.enter_context(nc.sbuf_tensor([P, NB * T * 3], F32))
        i_sb = ctx.enter_context(nc.sbuf_tensor([P, NB * T * 3], I32))
        m_sb = ctx.enter_context(nc.sbuf_tensor([P, NB * T * 2], U8))
        # per-buffer-slot DMA semaphores: HWDGE DMAs on different queues can
        # complete out of order, so a shared counter across slots would let a
        # consumer's wait be satisfied by the *other* slot's DMA.
        dmaA = [ctx.enter_context(nc.semaphore(f"dmaA{s}")) for s in range(NB)]
        dmaM = [ctx.enter_context(nc.semaphore(f"dmaM{s}")) for s in range(NB)]
        dmaOQ = [ctx.enter_context(nc.semaphore(f"dmaOQ{s}")) for s in range(NB)]
        dmaOI = [ctx.enter_context(nc.semaphore(f"dmaOI{s}")) for s in range(NB)]
        u_done = ctx.enter_context(nc.semaphore("u_done"))
        act_done = ctx.enter_context(nc.semaphore("act_done"))
        maskq_done = ctx.enter_context(nc.semaphore("maskq_done"))
        maski_done = ctx.enter_context(nc.semaphore("maski_done"))
        block = ctx.enter_context(nc.Block())

        def slot_rounds(j):  # (slot, dma-sem target) for chunk j
            return j % NB, 16 * (j // NB + 1)

        def a_view(j):  # [P, T, 3] f32 view of slot j%NB
            b = j % NB
            return a_sb[:, b * T * 3:(b + 1) * T * 3].rearrange(
                "p (t c) -> p t c", c=3)

        def i_view(j):
            b = j % NB
            return i_sb[:, b * T * 3:(b + 1) * T * 3].rearrange(
                "p (t c) -> p t c", c=3)

        def m_view(j):
            b = j % NB
            return m_sb[:, b * T * 2:(b + 1) * T * 2].rearrange(
                "p (t c) -> p t c", c=2)

        def a_flat(j):
            b = j % NB
            return a_sb[:, b * T * 3:(b + 1) * T * 3]

        def i_flat(j):
            b = j % NB
            return i_sb[:, b * T * 3:(b + 1) * T * 3]

        def m_flat(j):
            b = j % NB
            return m_sb[:, b * T * 2:(b + 1) * T * 2]

        @block.sync
        def _(sync):
            # loads only: the SP queue is in-order, so a store's wait on
            # compute progress here would stall *issuing* later loads and
            # put a per-chunk bubble in the DMA stream (measured ~6.5us).
            for j in range(N_CHUNKS):
                s, tgt = slot_rounds(j)
                if j >= NB:
                    # a_sb[s] free once the q out-DMA of chunk j-NB read it
                    sync.wait_ge(dmaOQ[s], tgt - 16)
                sync.dma_start(
                    a_flat(j), ang[:, j * T * 3:(j + 1) * T * 3]
                ).then_inc(dmaA[s], 16)
                if j >= NB:
                    # m_sb[s] free once the masking of chunk j-NB ran
                    sync.wait_ge(maski_done, j - NB + 1)
                sync.dma_start(
                    m_flat(j), msk[:, j * T * 2:(j + 1) * T * 2]
                ).then_inc(dmaM[s], 16)

        @block.gpsimd
        def _(gpsimd):
            # stores on the (otherwise idle) Pool queue
            for j in range(N_CHUNKS):
                s, tgt = slot_rounds(j)
                gpsimd.wait_ge(maskq_done, j + 1)
                gpsimd.dma_start(
                    oq[:, j * T * 3:(j + 1) * T * 3], a_flat(j)
                ).then_inc(dmaOQ[s], 16)
                gpsimd.wait_ge(maski_done, j + 1)
                gpsimd.dma_start(
                    oi[:, j * T * 3:(j + 1) * T * 3], i_flat(j)
                ).then_inc(dmaOI[s], 16)
            for s in range(NB):
                rounds = (N_CHUNKS + NB - 1 - s) // NB
                gpsimd.wait_ge(dmaOQ[s], 16 * rounds)
                gpsimd.wait_ge(dmaOI[s], 16 * rounds)

        @block.vector
        def _(vector):
            def u_pass(j):
                s, tgt = slot_rounds(j)
                vector.wait_ge(dmaA[s], tgt)
                av = a_view(j)
                for c in range(3):
                    ins = vector.tensor_scalar(
                        av[:, :, c], av[:, :, c],
                        float(_S[c]), float(_T[c]), ALU.mult, ALU.add)
                ins.then_inc(u_done, 1)

            def mask_pass(j):
                s, tgt = slot_rounds(j)
                vector.wait_ge(act_done, j + 1)
                vector.wait_ge(dmaM[s], tgt)
                qv, iv, mv = a_view(j), i_view(j), m_view(j)
                # q[...,c] *= (m == 0): exact q where unmasked, +-0 where
                # masked (host adds 0.0 to normalize -0).
                vector.scalar_tensor_tensor(
                    qv[:, :, 0], mv[:, :, 0], 0.0, qv[:, :, 0],
                    ALU.is_equal, ALU.mult)
                vector.scalar_tensor_tensor(
                    qv[:, :, 1], mv[:, :, 1], 0.0, qv[:, :, 1],
                    ALU.is_equal, ALU.mult).then_inc(maskq_done, 1)
                # i[...,c] = max(i, m * n_bins)
                vector.scalar_tensor_tensor(
                    iv[:, :, 0], mv[:, :, 0], float(N_BINS[0]), iv[:, :, 0],
                    ALU.mult, ALU.max)
                vector.scalar_tensor_tensor(
                    iv[:, :, 1], mv[:, :, 1], float(N_BINS[1]), iv[:, :, 1],
                    ALU.mult, ALU.max).then_inc(maski_done, 1)

            # software-pipelined: u'(j+1) is emitted before masks(j) so the
            # DVE never stalls on ACT inside one chunk's window.
            u_pass(0)
            for j in range(1, N_CHUNKS):
                u_pass(j)
                mask_pass(j - 1)
            mask_pass(N_CHUNKS - 1)

        @block.scalar
        def _(scalar):
            for j in range(N_CHUNKS):
                s, tgt = slot_rounds(j)
                scalar.wait_ge(u_done, j + 1)
                if j >= NB:
                    # i_sb[s] free once the idx out-DMA of chunk j-NB read it
                    scalar.wait_ge(dmaOI[s], tgt - 16)
                # i = rint(u'): ACT output convert f32->i32 rounds to nearest
                scalar.activation(i_flat(j), a_flat(j), ACT_COPY,
                                  bias=0.0, scale=1.0)
                # same-engine RAW: the centers read i_sb right behind the
                # cast's write; ACT is deep-pipelined, so drain in between.
                scalar.drain()
                iv, qv = i_view(j), a_view(j)
                # centers[i] = i*w + b (FMA), overwrites the dead u' tile
                for c in range(3):
                    ins = scalar.activation(
                        qv[:, :, c], iv[:, :, c], ACT_COPY,
                        bias=float(_B[c]), scale=float(_W[c]))
                ins.then_inc(act_done, 1)

    return nc


def _get_nc():
    global _NC_CACHE
    if _NC_CACHE is None:
        _NC_CACHE = _build_nc()
    return _NC_CACHE


# ---------------------------------------------------------------- host patch
def _centers_f32(n):
    k = np.arange(n, dtype=np.float32) + np.float32(0.5)
    return np.float32(-np.pi) + np.float32(2 * np.pi / n) * k


def _patch_boundaries(angles, null_mask, q_out, i_out):
    """Recompute exact reference semantics for elements within _PATCH_DELTA of
    an ideal bin boundary (f32 distance argmin, first-min tie break)."""
    TWO_PI = np.float32(2 * np.pi)
    a2 = angles.reshape(-1, 3)
    m2 = null_mask.reshape(-1, 2)
    q2 = q_out.reshape(-1, 3)
    i2 = i_out.reshape(-1, 3)
    for ch, n in enumerate(N_BINS):
        a = a2[:, ch]
        w = 2 * np.pi / n
        b = (a.astype(np.float64) + np.pi) / w
        near = np.abs(b - np.rint(b)) * w < _PATCH_DELTA
        if not np.any(near):
            continue
        af = a[near]
        centers = _centers_f32(n)
        diff = np.abs(af[:, None] - centers[None, :])
        dists = np.minimum(diff, TWO_PI - diff)
        idx = np.argmin(dists, axis=1).astype(np.int32)
        q = af + (centers[idx] - af)
        if ch < 2:
            m = m2[:, ch][near]
            q = np.where(m, np.float32(0.0), q)
            idx = np.where(m, np.int32(n), idx)
        q2[near, ch] = q
        i2[near, ch] = idx


# ---------------------------------------------------------------- entrypoint
def kernel(angles, null_mask):
    angles = np.asarray(angles, dtype=np.float32)
    null_mask = np.asarray(null_mask)
    assert angles.shape == (B0, B1, 3), angles.shape
    assert null_mask.shape == (B0, B1, 2), null_mask.shape
    mask_u8 = null_mask.view(np.uint8) if null_mask.dtype == np.bool_ \
        else null_mask.astype(np.uint8)

    nc = _get_nc()
    in_maps = []
    for c in range(N_CORES):
        sl = slice(c * ROWS_PER_CORE, (c + 1) * ROWS_PER_CORE)
        in_maps.append({
            "angles": np.ascontiguousarray(angles[sl]).reshape(P, -1),
            "null_mask": np.ascontiguousarray(mask_u8[sl]).reshape(P, -1),
        })

    results = run_bass_kernel_spmd(nc, in_maps, list(range(N_CORES))).results

    q_out = np.empty((B0, B1, 3), np.float32)
    i_out = np.empty((B0, B1, 3), np.int32)
    for c in range(N_CORES):
        sl = slice(c * ROWS_PER_CORE, (c + 1) * ROWS_PER_CORE)
        q_out[sl] = results[c]["q"].reshape(ROWS_PER_CORE, B1, 3)
        i_out[sl] = results[c]["idx"].reshape(ROWS_PER_CORE, B1, 3)

    np.add(q_out, np.float32(0.0), out=q_out)  # -0.0 -> +0.0 at masked slots
    _patch_boundaries(angles, np.asarray(null_mask, dtype=bool), q_out, i_out)
    return q_out, i_out


# revision 22
# speedup vs baseline: 1.0006x; 1.0006x over previous
"""CyclicVQ forward for Trainium2 (Bass, raw multi-engine pipeline, 8 cores).

Math: for each of 3 channels with n bins uniformly covering [-pi, pi), the
geodesic argmin over bin centers reduces to idx = rint(a*s + t) with
s = n/(2*pi), t = pi*s - 0.5 (f32 two-RN, matching the reference's decision
boundaries to within ~1 ulp).  quantized = centers[idx] via a fused ACT
affine (FMA) from the int index tile.  Null masking is fused
scalar_tensor_tensor ops: q *= (m == 0), i = max(i, m * n_bins).
A tiny host-side patch recomputes the exact reference semantics (f32
distance argmin) for the ~2k elements within 2e-5 of an ideal bin boundary,
where ulp-level rounding differences between the shortcut and the
reference's distance computation can flip the argmin.  A host `q += 0.0`
normalizes the -0.0 produced by masking negative q values.

Per-core pipeline (memory-bound; DMA ~13.6us per 1024-position chunk):
  SP:  load angles/mask chunks, store q/idx chunks (per-slot DMA sems)
  DVE: u' = a*s + t (3 strided fused TS), then masking (4 strided STT)
  ACT: i = rint(u') (contiguous convert), q = i*w + b (3 strided FMA)

Sharding: pure data parallel over the leading batch dim (4096 -> 8 x 512).
"""
import sys

sys.path.insert(0, "/opt/trn_rl_repo")

from contextlib import ExitStack

import numpy as np

import concourse.bass as bass
import concourse.mybir as mybir
from concourse.bass_utils import run_bass_kernel_spmd

# ---------------------------------------------------------------- constants
N_BINS = (24, 12, 16)
N_CORES = 8
B0, B1, B2 = 4096, 2048, 3  # angles shape
ROWS_PER_CORE = B0 // N_CORES  # 512
POS_PER_CORE = ROWS_PER_CORE * B1  # 1,048,576 positions
P = 128  # partitions
POS_PER_PART = POS_PER_CORE // P  # 8192
N_CHUNKS = 8
T = POS_PER_PART // N_CHUNKS  # 1024 positions / partition / chunk
NB = 4  # buffer slots (26KB SBUF per slot; 4 slots decouple load/store
        # by ~4 chunks, well past the ~25us per-chunk pipeline latency)

F32 = mybir.dt.float32
I32 = mybir.dt.int32
U8 = mybir.dt.uint8
ALU = mybir.AluOpType
ACT_COPY = mybir.ActivationFunctionType.Copy

_PI64 = np.float64(np.pi)
# per-channel device constants (f32, host-rounded)
_S = [np.float32(n / (2 * np.pi)) for n in N_BINS]  # u' = a*s + t
_T = [np.float32(_PI64 * np.float64(s) - 0.5) for n, s in zip(N_BINS, _S)]
_W = [np.float32(2 * np.pi / n) for n in N_BINS]  # center = i*w + b (FMA)
_B = [np.float32(0.5 * np.float64(w) - _PI64) for w in _W]

_PATCH_DELTA = 2e-5  # host-patch window around ideal boundaries (radians)

_NC_CACHE = None


def _build_nc():
    """Build the per-core Bass program (identical on all 8 cores)."""
    nc = bass.Bass()

    FE = POS_PER_PART * 3  # 24576 f32 per partition
    FM = POS_PER_PART * 2  # 16384 u8 per partition

    ang = nc.dram_tensor("angles", [P, FE], F32, kind="ExternalInput")
    msk = nc.dram_tensor("null_mask", [P, FM], U8, kind="ExternalInput")
    oq = nc.dram_tensor("q", [P, FE], F32, kind="ExternalOutput")
    oi = nc.dram_tensor("idx", [P, FE], I32, kind="ExternalOutput")

    with ExitStack() as ctx:
        # a_sb holds angles, then u' in place, then q (ACT writes centers
        # over the dead u') -- one f32 tile per slot instead of two.
        a_sb = ctx.enter_context(nc.sbuf_tensor([P, NB * T * 3], F32))
        i_sb = ctx.enter_context(nc.sbuf_tensor([P, NB * T * 3], I32))
        m_sb = ctx.enter_context(nc.sbuf_tensor([P, NB * T * 2], U8))
        # per-buffer-slot DMA semaphores: HWDGE DMAs on different queues can
        # complete out of order, so a shared counter across slots would let a
        # consumer's wait be satisfied by the *other* slot's DMA.
        dmaA = [ctx.enter_context(nc.semaphore(f"dmaA{s}")) for s in range(NB)]
        dmaM = [ctx.enter_context(nc.semaphore(f"dmaM{s}")) for s in range(NB)]
        dmaOQ = [ctx.enter_context(nc.semaphore(f"dmaOQ{s}")) for s in range(NB)]
        dmaOI = [ctx.enter_context(nc.semaphore(f"dmaOI{s}")) for s in range(NB)]
        u_done = ctx.enter_context(nc.semaphore("u_done"))
        act_done = ctx.enter_context(nc.semaphore("act_done"))
        maskq_done = ctx.enter_context(nc.semaphore("maskq_done"))
        maski_done = ctx.enter_context(nc.semaphore("maski_done"))
        block = ctx.enter_context(nc.Block())

        def slot_rounds(j):  # (slot, dma-sem target) for chunk j
            return j % NB, 16 * (j // NB + 1)

        def a_view(j):  # [P, T, 3] f32 view of slot j%NB
            b = j % NB
            return a_sb[:, b * T * 3:(b + 1) * T * 3].rearrange(
                "p (t c) -> p t c", c=3)

        def i_view(j):
            b = j % NB
            return i_sb[:, b * T * 3:(b + 1) * T * 3].rearrange(
                "p (t c) -> p t c", c=3)

        def m_view(j):
            b = j % NB
            return m_sb[:, b * T * 2:(b + 1) * T * 2].rearrange(
                "p (t c) -> p t c", c=2)

        def a_flat(j):
            b = j % NB
            return a_sb[:, b * T * 3:(b + 1) * T * 3]

        def i_flat(j):
            b = j % NB
            return i_sb[:, b * T * 3:(b + 1) * T * 3]

        def m_flat(j):
            b = j % NB
            return m_sb[:, b * T * 2:(b + 1) * T * 2]

        @block.sync
        def _(sync):
            # loads only: the SP queue is in-order, so a store's wait on
            # compute progress here would stall *issuing* later loads and
            # put a per-chunk bubble in the DMA stream (measured ~6.5us).
            for j in range(N_CHUNKS):
                s, tgt = slot_rounds(j)
                if j >= NB:
                    # a_sb[s] free once the q out-DMA of chunk j-NB read it
                    sync.wait_ge(dmaOQ[s], tgt - 16)
                sync.dma_start(
                    a_flat(j), ang[:, j * T * 3:(j + 1) * T * 3]
                ).then_inc(dmaA[s], 16)
                if j >= NB:
                    # m_sb[s] free once the masking of chunk j-NB ran
                    sync.wait_ge(maski_done, j - NB + 1)
                sync.dma_start(
                    m_flat(j), msk[:, j * T * 2:(j + 1) * T * 2]
                ).then_inc(dmaM[s], 16)

        @block.gpsimd
        def _(gpsimd):
            # stores on the (otherwise idle) Pool queue
            for j in range(N_CHUNKS):
                s, tgt = slot_rounds(j)
                gpsimd.wait_ge(maskq_done, j + 1)
                gpsimd.dma_start(
                    oq[:, j * T * 3:(j + 1) * T * 3], a_flat(j)
                ).then_inc(dmaOQ[s], 16)
                gpsimd.wait_ge(maski_done, j + 1)
                gpsimd.dma_start(
                    oi[:, j * T * 3:(j + 1) * T * 3], i_flat(j)
                ).then_inc(dmaOI[s], 16)
            for s in range(NB):
                rounds = (N_CHUNKS + NB - 1 - s) // NB
                gpsimd.wait_ge(dmaOQ[s], 16 * rounds)
                gpsimd.wait_ge(dmaOI[s], 16 * rounds)

        @block.vector
        def _(vector):
            def u_pass(j):
                s, tgt = slot_rounds(j)
                vector.wait_ge(dmaA[s], tgt)
                av = a_view(j)
                for c in range(3):
                    ins = vector.tensor_scalar(
                        av[:, :, c], av[:, :, c],
                        float(_S[c]), float(_T[c]), ALU.mult, ALU.add)
                ins.then_inc(u_done, 1)

            def mask_pass(j):
                s, tgt = slot_rounds(j)
                vector.wait_ge(act_done, j + 1)
                vector.wait_ge(dmaM[s], tgt)
                qv, iv, mv = a_view(j), i_view(j), m_view(j)
                # q[...,c] *= (m == 0): exact q where unmasked, +-0 where
                # masked (host adds 0.0 to normalize -0).
                vector.scalar_tensor_tensor(
                    qv[:, :, 0], mv[:, :, 0], 0.0, qv[:, :, 0],
                    ALU.is_equal, ALU.mult)
                vector.scalar_tensor_tensor(
                    qv[:, :, 1], mv[:, :, 1], 0.0, qv[:, :, 1],
                    ALU.is_equal, ALU.mult).then_inc(maskq_done, 1)
                # i[...,c] = max(i, m * n_bins)
                vector.scalar_tensor_tensor(
                    iv[:, :, 0], mv[:, :, 0], float(N_BINS[0]), iv[:, :, 0],
                    ALU.mult, ALU.max)
                vector.scalar_tensor_tensor(
                    iv[:, :, 1], mv[:, :, 1], float(N_BINS[1]), iv[:, :, 1],
                    ALU.mult, ALU.max).then_inc(maski_done, 1)

            # software-pipelined: u'(j+1) is emitted before masks(j) so the
            # DVE never stalls on ACT inside one chunk's window.
            u_pass(0)
            for j in range(1, N_CHUNKS):
                u_pass(j)
                mask_pass(j - 1)
            mask_pass(N_CHUNKS - 1)

        @block.scalar
        def _(scalar):
            for j in range(N_CHUNKS):
                s, tgt = slot_rounds(j)
                scalar.wait_ge(u_done, j + 1)
                if j >= NB:
                    # i_sb[s] free once the idx out-DMA of chunk j-NB read it
                    scalar.wait_ge(dmaOI[s], tgt - 16)
                # i = rint(u'): ACT output convert f32->i32 rounds to nearest
                scalar.activation(i_flat(j), a_flat(j), ACT_COPY,
                                  bias=0.0, scale=1.0)
                # same-engine RAW: the centers read i_sb right behind the
                # cast's write; ACT is deep-pipelined, so drain in between.
                scalar.drain()
                iv, qv = i_view(j), a_view(j)
                # centers[i] = i*w + b (FMA), overwrites the dead u' tile
                for c in range(3):
                    ins = scalar.activation(
                        qv[:, :, c], iv[:, :, c], ACT_COPY,
                        bias=float(_B[c]), scale=float(_W[c]))
                ins.then_inc(act_done, 1)

    return nc


def _get_nc():
    global _NC_CACHE
    if _NC_CACHE is None:
        _NC_CACHE = _build_nc()
    return _NC_CACHE


# ---------------------------------------------------------------- host patch
def _centers_f32(n):
    k = np.arange(n, dtype=np.float32) + np.float32(0.5)
    return np.float32(-np.pi) + np.float32(2 * np.pi / n) * k


def _patch_boundaries(angles, null_mask, q_out, i_out):
    """Recompute exact reference semantics for elements within _PATCH_DELTA of
    an ideal bin boundary (f32 distance argmin, first-min tie break)."""
    TWO_PI = np.float32(2 * np.pi)
    a2 = angles.reshape(-1, 3)
    m2 = null_mask.reshape(-1, 2)
    q2 = q_out.reshape(-1, 3)
    i2 = i_out.reshape(-1, 3)
    for ch, n in enumerate(N_BINS):
        a = a2[:, ch]
        w = 2 * np.pi / n
        b = (a.astype(np.float64) + np.pi) / w
        near = np.abs(b - np.rint(b)) * w < _PATCH_DELTA
        if not np.any(near):
            continue
        af = a[near]
        centers = _centers_f32(n)
        diff = np.abs(af[:, None] - centers[None, :])
        dists = np.minimum(diff, TWO_PI - diff)
        idx = np.argmin(dists, axis=1).astype(np.int32)
        q = af + (centers[idx] - af)
        if ch < 2:
            m = m2[:, ch][near]
            q = np.where(m, np.float32(0.0), q)
            idx = np.where(m, np.int32(n), idx)
        q2[near, ch] = q
        i2[near, ch] = idx


# ---------------------------------------------------------------- entrypoint
def kernel(angles, null_mask):
    angles = np.asarray(angles, dtype=np.float32)
    null_mask = np.asarray(null_mask)
    assert angles.shape == (B0, B1, 3), angles.shape
    assert null_mask.shape == (B0, B1, 2), null_mask.shape
    mask_u8 = null_mask.view(np.uint8) if null_mask.dtype == np.bool_ \
        else null_mask.astype(np.uint8)

    nc = _get_nc()
    in_maps = []
    for c in range(N_CORES):
        sl = slice(c * ROWS_PER_CORE, (c + 1) * ROWS_PER_CORE)
        in_maps.append({
            "angles": np.ascontiguousarray(angles[sl]).reshape(P, -1),
            "null_mask": np.ascontiguousarray(mask_u8[sl]).reshape(P, -1),
        })

    results = run_bass_kernel_spmd(nc, in_maps, list(range(N_CORES))).results

    q_out = np.empty((B0, B1, 3), np.float32)
    i_out = np.empty((B0, B1, 3), np.int32)
    for c in range(N_CORES):
        sl = slice(c * ROWS_PER_CORE, (c + 1) * ROWS_PER_CORE)
        q_out[sl] = results[c]["q"].reshape(ROWS_PER_CORE, B1, 3)
        i_out[sl] = results[c]["idx"].reshape(ROWS_PER_CORE, B1, 3)

    np.add(q_out, np.float32(0.0), out=q_out)  # -0.0 -> +0.0 at masked slots
    _patch_boundaries(angles, np.asarray(null_mask, dtype=bool), q_out, i_out)
    return q_out, i_out
